# revision 38
# baseline (speedup 1.0000x reference)
"""Multi-head self-attention TRN2 kernel (8 NeuronCores, SPMD).

Problem: B=2, S=2048, D=1024, H=16 heads, Dk=64.
Sharding: core c handles batch b=c//4 and head group g=c%4 (4 heads).
Each core computes a partial output (its heads' contribution through the
row-sharded Wo); the host sums the 4 partials per batch and adds bo.

Math trick: softmax(where(mask==0,-1e9,S)) == mask*exp(S) / sum(mask*exp(S))
exactly, and scores ~ N(0,1) here so exp never overflows without max
subtraction.

Key structure (v3):
  - Heads processed in PAIRS (even head -> SBUF partitions 0-63 / PE row
    groups 0-1, odd head -> partitions 64-127 / row groups 2-3). The two
    Dk=64 scores matmuls of a pair target disjoint row groups and disjoint
    PSUM banks, so the PE runs them CONCURRENTLY -- 2x on the scores phase.
  - q processed in blocks of 512 (last block split 256+256 to shorten the
    serial tail). Scores psum tile [128, 2*qw] holds both heads; one ACT
    exp per (pair, kc) covers both.
  - Mask multiply merged across 2 kc chunks and both heads: one DVE
    tensor_mul of [128, 4*qw] per kc-pair, mask AP repeated across heads
    via a stride-0 dim.
  - attnV per head [65, qw] (ones column gives softmax denominator l),
    lagged one kc-pair behind the mult.
  - Block epilogue is split: psum ctx is drained to SBUF (cnu) right away
    so the next block's attnV can claim the psum bank; the reciprocal /
    broadcast / normalize chain is emitted inside the NEXT block's kc loop
    (finish closure) to keep it off the PE critical path.
"""

import os
import numpy as np
import ml_dtypes

import concourse.bass as bass
import concourse.tile as tile
from concourse import bacc, mybir
from concourse.bass_utils import run_bass_kernel_spmd

FP32 = mybir.dt.float32
BF16 = mybir.dt.bfloat16
AF = mybir.ActivationFunctionType
ALU = mybir.AluOpType

S = 2048          # sequence length
D = 1024          # model dim
HPC = 4           # heads per core
DK = 64           # head dim
OC = HPC * DK     # 256 output dims per core for q/k/v
MT = D // 128     # 8 contraction chunks for projections
KC = S // 128     # 16 key chunks
NB = 512          # matmul moving-operand block

_NC_CACHE = None
LAST_RESULTS = None


def build_nc():
    nc = bacc.Bacc()

    qt_d = nc.dram_tensor("qt", [D, S], BF16, kind="ExternalInput")
    mask_d = nc.dram_tensor("maskt", [S, S], BF16, kind="ExternalInput")
    wq_d = nc.dram_tensor("wq", [D, OC], BF16, kind="ExternalInput")
    wk_d = nc.dram_tensor("wk", [D, OC], BF16, kind="ExternalInput")
    wv_d = nc.dram_tensor("wv", [D, OC], BF16, kind="ExternalInput")
    wo_d = nc.dram_tensor("wo", [OC, D], BF16, kind="ExternalInput")
    bq_d = nc.dram_tensor("bq8", [OC, 1], FP32, kind="ExternalInput")
    bk_d = nc.dram_tensor("bk1", [OC, 1], FP32, kind="ExternalInput")
    bv_d = nc.dram_tensor("bv1", [1, OC], BF16, kind="ExternalInput")
    out_d = nc.dram_tensor("out", [S, D], FP32, kind="ExternalOutput")

    with tile.TileContext(nc) as tc:
        from contextlib import ExitStack

        with ExitStack() as ctx:
            const = ctx.enter_context(tc.tile_pool(name="const", bufs=1))
            pexp = ctx.enter_context(tc.tile_pool(name="pexp", bufs=2))
            ppm = ctx.enter_context(tc.tile_pool(name="ppm", bufs=2))
            pcnu = ctx.enter_context(tc.tile_pool(name="pcnu", bufs=2))
            psmall = ctx.enter_context(tc.tile_pool(name="psmall", bufs=2))
            prb = ctx.enter_context(tc.tile_pool(name="prb", bufs=2))
            pcn = ctx.enter_context(tc.tile_pool(name="pcn", bufs=2))
            pout = ctx.enter_context(tc.tile_pool(name="pout", bufs=3))
            pdram = ctx.enter_context(
                tc.tile_pool(name="pdram", bufs=2, space="DRAM")
            )
            psA = ctx.enter_context(tc.tile_pool(name="psA", bufs=2, space="PSUM"))
            psS = ctx.enter_context(tc.tile_pool(name="psS", bufs=2, space="PSUM"))
            psC = ctx.enter_context(tc.tile_pool(name="psC", bufs=1, space="PSUM"))

            # ---------------- constant loads ----------------
            # wk (by t-chunk) + qt first: the prologue k-projection needs
            # only those. Everything else follows.
            wq_sb = const.tile([128, MT, OC], BF16)
            wk_sb = const.tile([128, MT, OC], BF16)
            wv_sb = const.tile([128, MT, OC], BF16)
            qt_sb = const.tile([128, MT, S], BF16)
            qtr = qt_d[:, :].rearrange(
                "(t p) (h c) -> t h p c", p=128, h=2
            )
            wkr = wk_d[:, :].rearrange("(t p) o -> t p o", p=128)
            # tiny biases FIRST -- the kT/qT psum->SBUF copies need them and
            # they must not queue behind the big strided weight gathers.
            bq_sb = const.tile([128, 2], FP32)
            bk_sb = const.tile([128, 2], FP32)
            bv_sb = const.tile([1, OC], BF16)
            bqr = bq_d[:, :].rearrange("(o p) u -> o p u", p=128)
            bkr = bk_d[:, :].rearrange("(o p) u -> o p u", p=128)
            for t in range(MT):
                nc.sync.dma_start(out=wk_sb[:, t, :], in_=wkr[t])
                (nc.gpsimd if t % 2 == 0 else nc.scalar).dma_start(
                    out=qt_sb[:, t, 0 : S // 2], in_=qtr[t, 0]
                )
            for o in range(2):
                nc.sync.dma_start(out=bk_sb[:, o : o + 1], in_=bkr[o])
                nc.sync.dma_start(out=bq_sb[:, o : o + 1], in_=bqr[o])
            nc.sync.dma_start(out=bv_sb, in_=bv_d[:, :])
            for t in range(MT):
                (nc.gpsimd if t % 2 == 0 else nc.scalar).dma_start(
                    out=qt_sb[:, t, S // 2 : S], in_=qtr[t, 1]
                )
            nc.sync.dma_start(
                out=wv_sb, in_=wv_d[:, :].rearrange("(t p) o -> p t o", p=128)
            )
            nc.gpsimd.dma_start(
                out=wq_sb, in_=wq_d[:, :].rearrange("(t p) o -> p t o", p=128)
            )
            wo_sb = const.tile([128, 2, D], BF16)
            nc.scalar.dma_start(
                out=wo_sb, in_=wo_d[:, :].rearrange("(i p) n -> p i n", p=128)
            )

            ones1 = const.tile([1, 128], BF16)
            nc.vector.memset(ones1, 1.0)
            # pre-warm the exp table-set while ACT is otherwise idle
            warm = const.tile([1, 128], BF16)
            nc.scalar.activation(out=warm, in_=ones1, func=AF.Exp)

            qT_sb = const.tile([128, 2, S], BF16)
            kT_sb = const.tile([128, 2, S], BF16)
            v_sb = const.tile([128, KC, HPC, DK + 1], BF16)
            nc.vector.memset(v_sb[:, :, :, DK : DK + 1], 1.0)
            ctxT_sb = [
                const.tile([128, S], BF16, name=f"ctxT{hb}", tag=f"ctxT{hb}")
                for hb in range(2)
            ]

            mask_sb = const.tile([128, KC, S], BF16)
            mr = mask_d[:, :].rearrange("(t p) s -> t p s", p=128)
            for t in range(2):
                (nc.sync if t % 2 == 0 else nc.gpsimd).dma_start(
                    out=mask_sb[:, t, :], in_=mr[t]
                )
            mask_loaded = 2

            def drip_mask(n=1):
                nonlocal mask_loaded
                for _ in range(n):
                    if mask_loaded < KC:
                        t = mask_loaded
                        (nc.sync if t % 2 == 0 else nc.gpsimd).dma_start(
                            out=mask_sb[:, t, :], in_=mr[t]
                        )
                        mask_loaded += 1

            # ---------------- projections ----------------
            # t=0 full-width (start=True settles has_written for the whole
            # bank), t>=1 as two concurrent 64-col col-tiles: halved
            # LDWEIGHTS and overlapped streams.
            def emit_proj_chain(pp, w_sb, osl, nsl):
                nc.tensor.matmul(
                    pp,
                    lhsT=w_sb[:, 0, osl],
                    rhs=qt_sb[:, 0, nsl],
                    start=True,
                    stop=False,
                )
                for t in range(1, MT):
                    for ph in (0, 64):
                        nc.tensor.matmul(
                            pp[ph : ph + 64, :],
                            lhsT=w_sb[:, t, osl.start + ph : osl.start + ph + 64],
                            rhs=qt_sb[:, t, nsl],
                            start=False,
                            stop=(t == MT - 1),
                        )

            def emit_qk_proj(ob, nbs=None, which="qk"):
                osl = slice(ob * 128, (ob + 1) * 128)
                for nb in nbs if nbs is not None else range(S // NB):
                    nsl = slice(nb * NB, (nb + 1) * NB)
                    if "q" in which:
                        ppq = psA.tile([128, NB], FP32, tag="ps512", name="ppq")
                        emit_proj_chain(ppq, wq_sb, osl, nsl)
                        # q' = (psum + bq)/8 ; host pre-divided bq by 8.
                        nc.vector.tensor_scalar(
                            out=qT_sb[:, ob, nsl],
                            in0=ppq,
                            scalar1=0.125,
                            scalar2=bq_sb[:, ob : ob + 1],
                            op0=ALU.mult,
                            op1=ALU.add,
                        )
                    if "k" in which:
                        ppk = psA.tile([128, NB], FP32, tag="ps512", name="ppk")
                        emit_proj_chain(ppk, wk_sb, osl, nsl)
                        nc.vector.tensor_scalar(
                            out=kT_sb[:, ob, nsl],
                            in0=ppk,
                            scalar1=bk_sb[:, ob : ob + 1],
                            scalar2=None,
                            op0=ALU.add,
                        )

            def emit_k_prologue():
                # t-major over nb pairs: first matmul only needs wk[0]+qt[0],
                # remaining chunk DMAs hide under compute.
                for nbp in range(2):
                    pps = [
                        psA.tile([128, NB], FP32, tag="ps512", name=f"ppk{i}")
                        for i in range(2)
                    ]
                    for t in range(MT):
                        for i in range(2):
                            nb = 2 * nbp + i
                            nc.tensor.matmul(
                                pps[i],
                                lhsT=wk_sb[:, t, 0:128],
                                rhs=qt_sb[:, t, nb * NB : (nb + 1) * NB],
                                start=(t == 0),
                                stop=(t == MT - 1),
                            )
                    for i in range(2):
                        nb = 2 * nbp + i
                        nc.vector.tensor_scalar(
                            out=kT_sb[:, 0, nb * NB : (nb + 1) * NB],
                            in0=pps[i],
                            scalar1=bk_sb[:, 0:1],
                            scalar2=None,
                            op0=ALU.add,
                        )

            # v: [s, o] per 128-row s-chunk; bias added via rank-1 matmul.
            def emit_v_proj(scs=None):
                for sc in scs if scs is not None else range(KC):
                    ssl = slice(sc * 128, (sc + 1) * 128)
                    ppv = psA.tile([128, NB], FP32, tag="ps512", name="ppv")
                    nc.tensor.matmul(
                        ppv[:, 0:OC],
                        lhsT=qt_sb[:, 0, ssl],
                        rhs=wv_sb[:, 0, :],
                        start=True,
                        stop=False,
                    )
                    for t in range(1, MT):
                        for ph in (0, 64):
                            nc.tensor.matmul(
                                ppv[ph : ph + 64, 0:OC],
                                lhsT=qt_sb[:, t, ssl.start + ph : ssl.start + ph + 64],
                                rhs=wv_sb[:, t, :],
                                start=False,
                                stop=False,
                            )
                    nc.tensor.matmul(
                        ppv[:, 0:OC], lhsT=ones1, rhs=bv_sb,
                        start=False, stop=True,
                    )
                    nc.vector.tensor_copy(
                        out=v_sb[:, sc, :, 0:DK],
                        in_=ppv[:, 0:OC].rearrange("p (h d) -> p h d", h=HPC),
                    )

            # ---------------- attention (head pairs) ----------------
            def emit_attn_pair(hb, q0, qw, hook=None, split_last_mult=False):
                """Process head pair (2*hb, 2*hb+1) for q columns
                [q0, q0+qw). Returns a `finish` closure that emits the
                softmax normalization chain (call from the next block's
                hook, or immediately for the last block)."""
                pc0 = psC.tile([DK + 1, qw], FP32, name="pc0", tag="pc0",
                               padded_shape=[128, NB])
                pc1 = psC.tile([DK + 1, qw], FP32, name="pc1", tag="pc1",
                               padded_shape=[128, NB])
                pcs = [pc0, pc1]

                def S_pair(kc):
                    # [128, 2, NB] so each head's half starts on a PSUM bank
                    # boundary even when qw < NB -- the two concurrent
                    # matmuls must never write the same 2KB bank.
                    ps = psS.tile([128, 2, NB], FP32, name="ps", tag="ps")
                    ksl = slice(kc * 128, (kc + 1) * 128)
                    for h in range(2):
                        hp = h * DK
                        nc.tensor.matmul(
                            ps[:, h, 0:qw],
                            lhsT=kT_sb[hp : hp + DK, hb, ksl],
                            rhs=qT_sb[hp : hp + DK, hb, q0 : q0 + qw],
                            start=True,
                            stop=True,
                        )
                    return ps

                def mask_rep(kc0, nkc):
                    mbase = mask_sb[:, kc0 : kc0 + nkc, q0 : q0 + qw]
                    return bass.AP(
                        tensor=mbase.tensor,
                        offset=mbase.offset,
                        ap=[
                            list(mbase.ap[0]),
                            list(mbase.ap[1]),
                            [0, 2],
                            list(mbase.ap[2]),
                        ],
                    )

                def attn_v(kc, pm, kci):
                    for h in range(2):
                        nc.tensor.matmul(
                            pcs[h],
                            lhsT=v_sb[:, kc, 2 * hb + h, :],
                            rhs=pm[:, kci, h, :],
                            start=(kc == 0),
                            stop=(kc == KC - 1),
                        )

                ps_cur = S_pair(0)
                pe = None
                pm_tiles = [None] * KC
                for kc in range(KC):
                    ps_next = S_pair(kc + 1) if kc + 1 < KC else None
                    if kc % 2 == 0:
                        pe = pexp.tile([128, 2, 2 * qw], BF16, tag="pe",
                                       name="pe", padded_shape=[128, 2, 2 * NB])
                    nc.scalar.activation(
                        out=pe[:, kc % 2, :].rearrange("p (b q) -> p b q", b=2),
                        in_=ps_cur[:, :, 0:qw],
                        func=AF.Exp,
                    )
                    last_pair = kc == KC - 1
                    if split_last_mult and kc >= KC - 2:
                        # single-kc mult+attnV to shorten the tail chain
                        pm = ppm.tile([128, 2, 2, qw], BF16, tag="pm",
                                      name="pm", padded_shape=[128, 2, 2, NB])
                        nc.vector.tensor_mul(
                            pm[:, 0:1, :, :],
                            pe[:, kc % 2 : kc % 2 + 1, :].rearrange(
                                "p a (b q) -> p a b q", b=2
                            ),
                            mask_rep(kc, 1),
                        )
                        if kc == KC - 2:
                            attn_v(KC - 4, pm_tiles[KC // 2 - 2], 0)
                            attn_v(KC - 3, pm_tiles[KC // 2 - 2], 1)
                        attn_v(kc, pm, 0)
                    elif kc % 2 == 1:
                        j = kc // 2
                        pm = ppm.tile([128, 2, 2, qw], BF16, tag="pm",
                                      name="pm", padded_shape=[128, 2, 2, NB])
                        nc.vector.tensor_mul(
                            pm,
                            pe.rearrange("p a (b q) -> p a b q", b=2),
                            mask_rep(2 * j, 2),
                        )
                        pm_tiles[j] = pm
                        if j >= 1:
                            attn_v(2 * (j - 1), pm_tiles[j - 1], 0)
                            attn_v(2 * (j - 1) + 1, pm_tiles[j - 1], 1)
                    ps_cur = ps_next
                    if hook is not None:
                        hook(kc)

                cnus = []

                def flush():
                    # deferred into the NEXT block's kc==0 hook so its
                    # dependency chain doesn't head-of-line-block the next
                    # block's scores matmuls in the PE queue.
                    if not split_last_mult:
                        attn_v(KC - 2, pm_tiles[KC // 2 - 1], 0)
                        attn_v(KC - 1, pm_tiles[KC // 2 - 1], 1)
                    # quick psum drain so the next block can claim the banks
                    for h in range(2):
                        cnu = pcnu.tile([DK + 1, qw], BF16, name=f"cnu{h}",
                                        tag=f"cnu{h}",
                                        padded_shape=[DK + 1, NB])
                        nc.vector.tensor_copy(out=cnu, in_=pcs[h])
                        cnus.append(cnu)

                def finish():
                    for h in range(2):
                        cnu = cnus[h]
                        hp = h * DK
                        # spread l over 128 partitions so the (multi-pass)
                        # reciprocal is cheap, then DRAM-bounce broadcast.
                        lw = psmall.tile([128, qw // 128], BF16, tag="lw",
                                         padded_shape=[128, NB // 128])
                        nc.sync.dma_start(out=lw, in_=cnu[DK : DK + 1, :])
                        lr = psmall.tile([128, qw // 128], BF16, tag="lr",
                                         padded_shape=[128, NB // 128])
                        with nc.allow_low_precision("softmax normalizer bf16"):
                            nc.vector.reciprocal(out=lr, in_=lw)
                        lr_dram = pdram.tile([1, qw], BF16,
                                             padded_shape=[1, NB])
                        nc.sync.dma_start(out=lr_dram, in_=lr)
                        rb = prb.tile([DK, qw], BF16, tag="rb",
                                      padded_shape=[DK, NB])
                        nc.sync.dma_start(
                            out=rb,
                            in_=bass.AP(
                                tensor=lr_dram.tensor, offset=lr_dram.offset,
                                ap=[[0, DK]] + list(lr_dram[:, :].ap[1:]),
                            ),
                        )
                        if hp == 0:
                            nc.vector.tensor_mul(
                                ctxT_sb[hb][0:DK, q0 : q0 + qw],
                                cnu[0:DK, :], rb,
                            )
                        else:
                            cn = pcn.tile([DK, qw], BF16, tag="cn",
                                          padded_shape=[DK, NB])
                            nc.vector.tensor_mul(cn, cnu[0:DK, :], rb)
                            nc.gpsimd.dma_start(
                                out=ctxT_sb[hb][hp : hp + DK, q0 : q0 + qw],
                                in_=cn,
                            )

                return flush, finish

            # ---------------- output projection ----------------
            outr = out_d[:, :].rearrange("(qc p) n -> qc p n", p=128)

            def emit_outproj(qcs, copy_on_act=False):
                for qc in qcs:
                    ob_sb = pout.tile([128, D], FP32, name="ob_sb")
                    for nb in range(D // NB):
                        po = psA.tile([128, NB], FP32, tag="ps512", name="po")
                        nc.tensor.matmul(
                            po,
                            lhsT=ctxT_sb[0][:, qc * 128 : (qc + 1) * 128],
                            rhs=wo_sb[:, 0, nb * NB : (nb + 1) * NB],
                            start=True,
                            stop=False,
                        )
                        for ph in (0, 64):
                            nc.tensor.matmul(
                                po[ph : ph + 64, :],
                                lhsT=ctxT_sb[1][
                                    :, qc * 128 + ph : qc * 128 + ph + 64
                                ],
                                rhs=wo_sb[:, 1, nb * NB : (nb + 1) * NB],
                                start=False,
                                stop=True,
                            )
                        osl = slice(nb * NB, (nb + 1) * NB)
                        if copy_on_act:
                            nc.scalar.copy(out=ob_sb[:, osl], in_=po)
                        else:
                            nc.vector.tensor_copy(out=ob_sb[:, osl], in_=po)
                    nc.sync.dma_start(
                        out=outr[qc, :, 0 : D // 2], in_=ob_sb[:, 0 : D // 2]
                    )
                    nc.gpsimd.dma_start(
                        out=outr[qc, :, D // 2 : D], in_=ob_sb[:, D // 2 : D]
                    )

            # ---------------- emission schedule ----------------
            # Prologue: k ob0 (t-major, minimal DMA gating), q ob0 nb0,
            # first v chunks.
            emit_k_prologue()
            emit_qk_proj(0, nbs=[0], which="q")
            emit_v_proj(scs=[0, 1, 2])

            from collections import deque

            drip = deque()
            flush_prev = None
            finish_prev = None

            def hook(kc):
                nonlocal flush_prev, finish_prev
                drip_mask(1)
                if kc == 0 and flush_prev is not None:
                    flush_prev()
                    flush_prev = None
                    return
                if kc == 1 and finish_prev is not None:
                    # previous block's normalization chain MUST be emitted
                    # before any dripped outproj that reads its ctxT.
                    finish_prev()
                    finish_prev = None
                    return
                if kc >= 2 and drip:
                    drip.popleft()()

            # blocks: (hb, q0, qw, drip thunks for this block)
            blocks = []
            blocks.append((0, 0, 512, (
                [lambda sc=sc: emit_v_proj(scs=[sc]) for sc in range(3, 10)]
                + [lambda: emit_qk_proj(0, nbs=[1], which="q")]
                + [lambda sc=sc: emit_v_proj(scs=[sc]) for sc in range(10, 16)]
            )))
            blocks.append((0, 512, 512, [
                lambda: emit_qk_proj(0, nbs=[2], which="q"),
            ]))
            blocks.append((0, 1024, 512, [
                lambda: emit_qk_proj(0, nbs=[3], which="q"),
                lambda: emit_qk_proj(1, nbs=[0], which="k"),
                lambda: emit_qk_proj(1, nbs=[1], which="k"),
            ]))
            blocks.append((0, 1536, 512, [
                lambda: emit_qk_proj(1, nbs=[2], which="k"),
                lambda: emit_qk_proj(1, nbs=[3], which="k"),
                lambda: emit_qk_proj(1, nbs=[0], which="q"),
            ]))
            blocks.append((1, 0, 512, [
                lambda: emit_qk_proj(1, nbs=[1], which="q"),
            ]))
            blocks.append((1, 512, 512, [
                lambda: emit_qk_proj(1, nbs=[2], which="q"),
                lambda: emit_outproj([0, 1]),
                lambda: emit_outproj([2, 3]),
            ]))
            blocks.append((1, 1024, 512, [
                lambda: emit_qk_proj(1, nbs=[3], which="q"),
                lambda: emit_outproj([4, 5]),
                lambda: emit_outproj([6, 7]),
            ]))
            blocks.append((1, 1536, 512, [
                lambda: emit_outproj([8, 9]),
                lambda: emit_outproj([10, 11]),
            ]))

            for bi, (hb, q0, qw, dr) in enumerate(blocks):
                drip.extend(dr)
                last = bi == len(blocks) - 1
                fl, fin = emit_attn_pair(hb, q0, qw, hook,
                                         split_last_mult=last)
                if last:
                    fl()
                    fin()
                else:
                    flush_prev = fl
                    finish_prev = fin
            emit_outproj([12], copy_on_act=True)
            emit_outproj([13], copy_on_act=True)
            emit_outproj([14], copy_on_act=True)
            emit_outproj([15], copy_on_act=True)

    nc.compile()
    return nc


def _get_nc():
    global _NC_CACHE
    if _NC_CACHE is None:
        _NC_CACHE = build_nc()
    return _NC_CACHE


def kernel(Q, attn_mask, Wq, bq, Wk, bk, Wv, bv, Wo, bo):
    global LAST_RESULTS
    bf16 = ml_dtypes.bfloat16
    Q = np.asarray(Q, np.float32)
    attn_mask = np.asarray(attn_mask)
    Wq, Wk, Wv, Wo = (np.asarray(w, np.float32) for w in (Wq, Wk, Wv, Wo))
    bq, bk, bv, bo = (np.asarray(b, np.float32) for b in (bq, bk, bv, bo))
    B = Q.shape[0]

    nc = _get_nc()
    in_maps = []
    for c in range(8):
        b, g = c // 4, c % 4
        hs = slice(OC * g, OC * (g + 1))
        in_maps.append(
            {
                "qt": np.ascontiguousarray(Q[b].T).astype(bf16),
                "maskt": np.ascontiguousarray(attn_mask[b, 0].T).astype(bf16),
                "wq": np.ascontiguousarray(Wq[hs].T).astype(bf16),
                "wk": np.ascontiguousarray(Wk[hs].T).astype(bf16),
                "wv": np.ascontiguousarray(Wv[hs].T).astype(bf16),
                "wo": np.ascontiguousarray(Wo[:, hs].T).astype(bf16),
                "bq8": (bq[hs] * 0.125).reshape(OC, 1).astype(np.float32),
                "bk1": bk[hs].reshape(OC, 1).astype(np.float32),
                "bv1": bv[hs].reshape(1, OC).astype(bf16),
            }
        )

    res = run_bass_kernel_spmd(
        nc, in_maps, core_ids=list(range(8)),
        trace=bool(int(os.environ.get("KERNEL_TRACE", "0"))),
    )
    LAST_RESULTS = res
    out = np.zeros((B, S, D), np.float32)
    for c in range(8):
        out[c // 4] += np.asarray(res.results[c]["out"], np.float32)
    out += bo
    return out


# revision 41
# speedup vs baseline: 1.0317x; 1.0317x over previous
"""Multi-head self-attention TRN2 kernel (8 NeuronCores, SPMD).

Problem: B=2, S=2048, D=1024, H=16 heads, Dk=64.
Sharding: core c handles batch b=c//4 and head group g=c%4 (4 heads).
Each core computes a partial output (its heads' contribution through the
row-sharded Wo); the host sums the 4 partials per batch and adds bo.

Math trick: softmax(where(mask==0,-1e9,S)) == mask*exp(S) / sum(mask*exp(S))
exactly, and scores ~ N(0,1) here so exp never overflows without max
subtraction.

Key structure (v3):
  - Heads processed in PAIRS (even head -> SBUF partitions 0-63 / PE row
    groups 0-1, odd head -> partitions 64-127 / row groups 2-3). The two
    Dk=64 scores matmuls of a pair target disjoint row groups and disjoint
    PSUM banks, so the PE runs them CONCURRENTLY -- 2x on the scores phase.
  - q processed in blocks of 512 (last block split 256+256 to shorten the
    serial tail). Scores psum tile [128, 2*qw] holds both heads; one ACT
    exp per (pair, kc) covers both.
  - Mask multiply merged across 2 kc chunks and both heads: one DVE
    tensor_mul of [128, 4*qw] per kc-pair, mask AP repeated across heads
    via a stride-0 dim.
  - attnV per head [65, qw] (ones column gives softmax denominator l),
    lagged one kc-pair behind the mult.
  - Block epilogue is split: psum ctx is drained to SBUF (cnu) right away
    so the next block's attnV can claim the psum bank; the reciprocal /
    broadcast / normalize chain is emitted inside the NEXT block's kc loop
    (finish closure) to keep it off the PE critical path.
"""

import os
import numpy as np
import ml_dtypes

import concourse.bass as bass
import concourse.tile as tile
from concourse import bacc, mybir
from concourse.bass_utils import run_bass_kernel_spmd

FP32 = mybir.dt.float32
BF16 = mybir.dt.bfloat16
AF = mybir.ActivationFunctionType
ALU = mybir.AluOpType

S = 2048          # sequence length
D = 1024          # model dim
HPC = 4           # heads per core
DK = 64           # head dim
OC = HPC * DK     # 256 output dims per core for q/k/v
MT = D // 128     # 8 contraction chunks for projections
KC = S // 128     # 16 key chunks
NB = 512          # matmul moving-operand block

_NC_CACHE = None
LAST_RESULTS = None


def build_nc():
    nc = bacc.Bacc()

    qt_d = nc.dram_tensor("qt", [D, S], BF16, kind="ExternalInput")
    mask_d = nc.dram_tensor("maskt", [S, S], BF16, kind="ExternalInput")
    wq_d = nc.dram_tensor("wq", [D, OC], BF16, kind="ExternalInput")
    wk_d = nc.dram_tensor("wk", [D, OC], BF16, kind="ExternalInput")
    wv_d = nc.dram_tensor("wv", [D, OC], BF16, kind="ExternalInput")
    wo_d = nc.dram_tensor("wo", [OC, D], BF16, kind="ExternalInput")
    bq_d = nc.dram_tensor("bq8", [OC, 1], FP32, kind="ExternalInput")
    bk_d = nc.dram_tensor("bk1", [OC, 1], FP32, kind="ExternalInput")
    bv_d = nc.dram_tensor("bv1", [1, OC], BF16, kind="ExternalInput")
    out_d = nc.dram_tensor("out", [S, D], FP32, kind="ExternalOutput")

    with tile.TileContext(nc) as tc:
        from contextlib import ExitStack

        with ExitStack() as ctx:
            const = ctx.enter_context(tc.tile_pool(name="const", bufs=1))
            pexp = ctx.enter_context(tc.tile_pool(name="pexp", bufs=2))
            ppm = ctx.enter_context(tc.tile_pool(name="ppm", bufs=2))
            pcnu = ctx.enter_context(tc.tile_pool(name="pcnu", bufs=2))
            psmall = ctx.enter_context(tc.tile_pool(name="psmall", bufs=2))
            prb = ctx.enter_context(tc.tile_pool(name="prb", bufs=2))
            pcn = ctx.enter_context(tc.tile_pool(name="pcn", bufs=2))
            pout = ctx.enter_context(tc.tile_pool(name="pout", bufs=3))
            pdram = ctx.enter_context(
                tc.tile_pool(name="pdram", bufs=2, space="DRAM")
            )
            psA = ctx.enter_context(tc.tile_pool(name="psA", bufs=2, space="PSUM"))
            psS = ctx.enter_context(tc.tile_pool(name="psS", bufs=2, space="PSUM"))
            psC = ctx.enter_context(tc.tile_pool(name="psC", bufs=1, space="PSUM"))

            # ---------------- constant loads ----------------
            # wk (by t-chunk) + qt first: the prologue k-projection needs
            # only those. Everything else follows.
            wq_sb = const.tile([128, MT, OC], BF16)
            wk_sb = const.tile([128, MT, OC], BF16)
            wv_sb = const.tile([128, MT, OC], BF16)
            qt_sb = const.tile([128, MT, S], BF16)
            qtr = qt_d[:, :].rearrange(
                "(t p) (h c) -> t h p c", p=128, h=2
            )
            wkr = wk_d[:, :].rearrange("(t p) o -> t p o", p=128)
            # tiny biases FIRST -- the kT/qT psum->SBUF copies need them and
            # they must not queue behind the big strided weight gathers.
            bq_sb = const.tile([128, 2], FP32)
            bk_sb = const.tile([128, 2], FP32)
            bv_sb = const.tile([1, OC], BF16)
            bqr = bq_d[:, :].rearrange("(o p) u -> o p u", p=128)
            bkr = bk_d[:, :].rearrange("(o p) u -> o p u", p=128)
            for t in range(MT):
                nc.sync.dma_start(out=wk_sb[:, t, :], in_=wkr[t])
                (nc.gpsimd if t % 2 == 0 else nc.scalar).dma_start(
                    out=qt_sb[:, t, 0 : S // 2], in_=qtr[t, 0]
                )
            for o in range(2):
                nc.sync.dma_start(out=bk_sb[:, o : o + 1], in_=bkr[o])
                nc.sync.dma_start(out=bq_sb[:, o : o + 1], in_=bqr[o])
            nc.sync.dma_start(out=bv_sb, in_=bv_d[:, :])
            for t in range(MT):
                (nc.gpsimd if t % 2 == 0 else nc.scalar).dma_start(
                    out=qt_sb[:, t, S // 2 : S], in_=qtr[t, 1]
                )
            nc.sync.dma_start(
                out=wv_sb, in_=wv_d[:, :].rearrange("(t p) o -> p t o", p=128)
            )
            nc.gpsimd.dma_start(
                out=wq_sb, in_=wq_d[:, :].rearrange("(t p) o -> p t o", p=128)
            )
            wo_sb = const.tile([128, 2, D], BF16)
            nc.scalar.dma_start(
                out=wo_sb, in_=wo_d[:, :].rearrange("(i p) n -> p i n", p=128)
            )

            ones1 = const.tile([1, 128], BF16)
            nc.vector.memset(ones1, 1.0)
            # pre-warm the exp table-set while ACT is otherwise idle
            warm = const.tile([1, 128], BF16)
            nc.scalar.activation(out=warm, in_=ones1, func=AF.Exp)

            qT_sb = const.tile([128, 2, S], BF16)
            kT_sb = const.tile([128, 2, S], BF16)
            v_sb = const.tile([128, KC, HPC, DK + 1], BF16)
            nc.vector.memset(v_sb[:, :, :, DK : DK + 1], 1.0)
            ctxT_sb = [
                const.tile([128, S], BF16, name=f"ctxT{hb}", tag=f"ctxT{hb}")
                for hb in range(2)
            ]

            mask_sb = const.tile([128, KC, S], BF16)
            mr = mask_d[:, :].rearrange("(t p) s -> t p s", p=128)
            for t in range(2):
                (nc.sync if t % 2 == 0 else nc.gpsimd).dma_start(
                    out=mask_sb[:, t, :], in_=mr[t]
                )
            mask_loaded = 2

            def drip_mask(n=1):
                nonlocal mask_loaded
                for _ in range(n):
                    if mask_loaded < KC:
                        t = mask_loaded
                        (nc.sync if t % 2 == 0 else nc.gpsimd).dma_start(
                            out=mask_sb[:, t, :], in_=mr[t]
                        )
                        mask_loaded += 1

            # ---------------- projections ----------------
            # t=0 full-width (start=True settles has_written for the whole
            # bank), t>=1 as two concurrent 64-col col-tiles: halved
            # LDWEIGHTS and overlapped streams.
            def emit_proj_chain(pp, w_sb, osl, nsl):
                for t in range(MT):
                    nc.tensor.matmul(
                        pp,
                        lhsT=w_sb[:, t, osl],
                        rhs=qt_sb[:, t, nsl],
                        start=(t == 0),
                        stop=(t == MT - 1),
                    )

            def emit_qk_proj(ob, nbs=None, which="qk"):
                osl = slice(ob * 128, (ob + 1) * 128)
                for nb in nbs if nbs is not None else range(S // NB):
                    nsl = slice(nb * NB, (nb + 1) * NB)
                    if "q" in which:
                        ppq = psA.tile([128, NB], FP32, tag="ps512", name="ppq")
                        emit_proj_chain(ppq, wq_sb, osl, nsl)
                        # q' = (psum + bq)/8 ; host pre-divided bq by 8.
                        nc.vector.tensor_scalar(
                            out=qT_sb[:, ob, nsl],
                            in0=ppq,
                            scalar1=0.125,
                            scalar2=bq_sb[:, ob : ob + 1],
                            op0=ALU.mult,
                            op1=ALU.add,
                        )
                    if "k" in which:
                        ppk = psA.tile([128, NB], FP32, tag="ps512", name="ppk")
                        emit_proj_chain(ppk, wk_sb, osl, nsl)
                        nc.vector.tensor_scalar(
                            out=kT_sb[:, ob, nsl],
                            in0=ppk,
                            scalar1=bk_sb[:, ob : ob + 1],
                            scalar2=None,
                            op0=ALU.add,
                        )

            def emit_k_prologue():
                # t-major over nb pairs: first matmul only needs wk[0]+qt[0],
                # remaining chunk DMAs hide under compute.
                for nbp in range(2):
                    pps = [
                        psA.tile([128, NB], FP32, tag="ps512", name=f"ppk{i}")
                        for i in range(2)
                    ]
                    for t in range(MT):
                        for i in range(2):
                            nb = 2 * nbp + i
                            nc.tensor.matmul(
                                pps[i],
                                lhsT=wk_sb[:, t, 0:128],
                                rhs=qt_sb[:, t, nb * NB : (nb + 1) * NB],
                                start=(t == 0),
                                stop=(t == MT - 1),
                            )
                    for i in range(2):
                        nb = 2 * nbp + i
                        nc.vector.tensor_scalar(
                            out=kT_sb[:, 0, nb * NB : (nb + 1) * NB],
                            in0=pps[i],
                            scalar1=bk_sb[:, 0:1],
                            scalar2=None,
                            op0=ALU.add,
                        )

            # v: [s, o] per 128-row s-chunk; bias added via rank-1 matmul.
            def emit_v_proj(scs=None):
                for sc in scs if scs is not None else range(KC):
                    ssl = slice(sc * 128, (sc + 1) * 128)
                    ppv = psA.tile([128, NB], FP32, tag="ps512", name="ppv")
                    for t in range(MT):
                        nc.tensor.matmul(
                            ppv[:, 0:OC],
                            lhsT=qt_sb[:, t, ssl],
                            rhs=wv_sb[:, t, :],
                            start=(t == 0),
                            stop=False,
                        )
                    nc.tensor.matmul(
                        ppv[:, 0:OC], lhsT=ones1, rhs=bv_sb,
                        start=False, stop=True,
                    )
                    nc.vector.tensor_copy(
                        out=v_sb[:, sc, :, 0:DK],
                        in_=ppv[:, 0:OC].rearrange("p (h d) -> p h d", h=HPC),
                    )

            # ---------------- attention (head pairs) ----------------
            def emit_attn_pair(hb, q0, qw, hook=None, split_last_mult=False):
                """Process head pair (2*hb, 2*hb+1) for q columns
                [q0, q0+qw). Returns a `finish` closure that emits the
                softmax normalization chain (call from the next block's
                hook, or immediately for the last block)."""
                pc0 = psC.tile([DK + 1, qw], FP32, name="pc0", tag="pc0",
                               padded_shape=[128, NB])
                pc1 = psC.tile([DK + 1, qw], FP32, name="pc1", tag="pc1",
                               padded_shape=[128, NB])
                pcs = [pc0, pc1]

                def S_pair(kc):
                    # [128, 2, NB] so each head's half starts on a PSUM bank
                    # boundary even when qw < NB -- the two concurrent
                    # matmuls must never write the same 2KB bank.
                    ps = psS.tile([128, 2, NB], FP32, name="ps", tag="ps")
                    ksl = slice(kc * 128, (kc + 1) * 128)
                    for h in range(2):
                        hp = h * DK
                        nc.tensor.matmul(
                            ps[:, h, 0:qw],
                            lhsT=kT_sb[hp : hp + DK, hb, ksl],
                            rhs=qT_sb[hp : hp + DK, hb, q0 : q0 + qw],
                            start=True,
                            stop=True,
                        )
                    return ps

                def mask_rep(kc0, nkc):
                    mbase = mask_sb[:, kc0 : kc0 + nkc, q0 : q0 + qw]
                    return bass.AP(
                        tensor=mbase.tensor,
                        offset=mbase.offset,
                        ap=[
                            list(mbase.ap[0]),
                            list(mbase.ap[1]),
                            [0, 2],
                            list(mbase.ap[2]),
                        ],
                    )

                def attn_v(kc, pm, kci):
                    for h in range(2):
                        nc.tensor.matmul(
                            pcs[h],
                            lhsT=v_sb[:, kc, 2 * hb + h, :],
                            rhs=pm[:, kci, h, :],
                            start=(kc == 0),
                            stop=(kc == KC - 1),
                        )

                ps_cur = S_pair(0)
                pe = None
                pm_tiles = [None] * KC
                for kc in range(KC):
                    ps_next = S_pair(kc + 1) if kc + 1 < KC else None
                    if kc % 2 == 0:
                        pe = pexp.tile([128, 2, 2 * qw], BF16, tag="pe",
                                       name="pe", padded_shape=[128, 2, 2 * NB])
                    nc.scalar.activation(
                        out=pe[:, kc % 2, :].rearrange("p (b q) -> p b q", b=2),
                        in_=ps_cur[:, :, 0:qw],
                        func=AF.Exp,
                    )
                    last_pair = kc == KC - 1
                    if split_last_mult and kc >= KC - 2:
                        # single-kc mult+attnV to shorten the tail chain
                        pm = ppm.tile([128, 2, 2, qw], BF16, tag="pm",
                                      name="pm", padded_shape=[128, 2, 2, NB])
                        nc.vector.tensor_mul(
                            pm[:, 0:1, :, :],
                            pe[:, kc % 2 : kc % 2 + 1, :].rearrange(
                                "p a (b q) -> p a b q", b=2
                            ),
                            mask_rep(kc, 1),
                        )
                        if kc == KC - 2:
                            attn_v(KC - 4, pm_tiles[KC // 2 - 2], 0)
                            attn_v(KC - 3, pm_tiles[KC // 2 - 2], 1)
                        attn_v(kc, pm, 0)
                    elif kc % 2 == 1:
                        j = kc // 2
                        pm = ppm.tile([128, 2, 2, qw], BF16, tag="pm",
                                      name="pm", padded_shape=[128, 2, 2, NB])
                        nc.vector.tensor_mul(
                            pm,
                            pe.rearrange("p a (b q) -> p a b q", b=2),
                            mask_rep(2 * j, 2),
                        )
                        pm_tiles[j] = pm
                        if j >= 1:
                            attn_v(2 * (j - 1), pm_tiles[j - 1], 0)
                            attn_v(2 * (j - 1) + 1, pm_tiles[j - 1], 1)
                    ps_cur = ps_next
                    if hook is not None:
                        hook(kc)

                cnus = []

                def flush():
                    # deferred into the NEXT block's kc==0 hook so its
                    # dependency chain doesn't head-of-line-block the next
                    # block's scores matmuls in the PE queue.
                    if not split_last_mult:
                        attn_v(KC - 2, pm_tiles[KC // 2 - 1], 0)
                        attn_v(KC - 1, pm_tiles[KC // 2 - 1], 1)
                    # quick psum drain so the next block can claim the banks
                    for h in range(2):
                        cnu = pcnu.tile([DK + 1, qw], BF16, name=f"cnu{h}",
                                        tag=f"cnu{h}",
                                        padded_shape=[DK + 1, NB])
                        nc.vector.tensor_copy(out=cnu, in_=pcs[h])
                        cnus.append(cnu)

                def finish():
                    for h in range(2):
                        cnu = cnus[h]
                        hp = h * DK
                        # spread l over 128 partitions so the (multi-pass)
                        # reciprocal is cheap, then DRAM-bounce broadcast.
                        lw = psmall.tile([128, qw // 128], BF16, tag="lw",
                                         padded_shape=[128, NB // 128])
                        nc.sync.dma_start(out=lw, in_=cnu[DK : DK + 1, :])
                        lr = psmall.tile([128, qw // 128], BF16, tag="lr",
                                         padded_shape=[128, NB // 128])
                        with nc.allow_low_precision("softmax normalizer bf16"):
                            nc.vector.reciprocal(out=lr, in_=lw)
                        lr_dram = pdram.tile([1, qw], BF16,
                                             padded_shape=[1, NB])
                        nc.sync.dma_start(out=lr_dram, in_=lr)
                        rb = prb.tile([DK, qw], BF16, tag="rb",
                                      padded_shape=[DK, NB])
                        nc.sync.dma_start(
                            out=rb,
                            in_=bass.AP(
                                tensor=lr_dram.tensor, offset=lr_dram.offset,
                                ap=[[0, DK]] + list(lr_dram[:, :].ap[1:]),
                            ),
                        )
                        if hp == 0:
                            nc.vector.tensor_mul(
                                ctxT_sb[hb][0:DK, q0 : q0 + qw],
                                cnu[0:DK, :], rb,
                            )
                        else:
                            cn = pcn.tile([DK, qw], BF16, tag="cn",
                                          padded_shape=[DK, NB])
                            nc.vector.tensor_mul(cn, cnu[0:DK, :], rb)
                            nc.gpsimd.dma_start(
                                out=ctxT_sb[hb][hp : hp + DK, q0 : q0 + qw],
                                in_=cn,
                            )

                return flush, finish

            # ---------------- output projection ----------------
            outr = out_d[:, :].rearrange("(qc p) n -> qc p n", p=128)

            def emit_outproj(qcs, copy_on_act=False):
                for qc in qcs:
                    ob_sb = pout.tile([128, D], FP32, name="ob_sb")
                    for nb in range(D // NB):
                        po = psA.tile([128, NB], FP32, tag="ps512", name="po")
                        for ic in range(2):
                            nc.tensor.matmul(
                                po,
                                lhsT=ctxT_sb[ic][:, qc * 128 : (qc + 1) * 128],
                                rhs=wo_sb[:, ic, nb * NB : (nb + 1) * NB],
                                start=(ic == 0),
                                stop=(ic == 1),
                            )
                        osl = slice(nb * NB, (nb + 1) * NB)
                        if copy_on_act:
                            nc.scalar.copy(out=ob_sb[:, osl], in_=po)
                        else:
                            nc.vector.tensor_copy(out=ob_sb[:, osl], in_=po)
                    nc.sync.dma_start(
                        out=outr[qc, :, 0 : D // 2], in_=ob_sb[:, 0 : D // 2]
                    )
                    nc.gpsimd.dma_start(
                        out=outr[qc, :, D // 2 : D], in_=ob_sb[:, D // 2 : D]
                    )

            # ---------------- emission schedule ----------------
            # Prologue: k ob0 (t-major, minimal DMA gating), q ob0 nb0,
            # first v chunks.
            emit_k_prologue()
            emit_qk_proj(0, nbs=[0], which="q")
            emit_v_proj(scs=[0, 1, 2])

            from collections import deque

            drip = deque()
            flush_prev = None
            finish_prev = None

            def hook(kc):
                nonlocal flush_prev, finish_prev
                drip_mask(1)
                if kc == 0 and flush_prev is not None:
                    flush_prev()
                    flush_prev = None
                    return
                if kc == 1 and finish_prev is not None:
                    # previous block's normalization chain MUST be emitted
                    # before any dripped outproj that reads its ctxT.
                    finish_prev()
                    finish_prev = None
                    return
                if kc >= 2 and drip:
                    drip.popleft()()

            # blocks: (hb, q0, qw, drip thunks for this block)
            blocks = []
            blocks.append((0, 0, 512, (
                [lambda sc=sc: emit_v_proj(scs=[sc]) for sc in range(3, 10)]
                + [lambda: emit_qk_proj(0, nbs=[1], which="q")]
                + [lambda sc=sc: emit_v_proj(scs=[sc]) for sc in range(10, 16)]
            )))
            blocks.append((0, 512, 512, [
                lambda: emit_qk_proj(0, nbs=[2], which="q"),
            ]))
            blocks.append((0, 1024, 512, [
                lambda: emit_qk_proj(0, nbs=[3], which="q"),
                lambda: emit_qk_proj(1, nbs=[0], which="k"),
                lambda: emit_qk_proj(1, nbs=[1], which="k"),
            ]))
            blocks.append((0, 1536, 512, [
                lambda: emit_qk_proj(1, nbs=[2], which="k"),
                lambda: emit_qk_proj(1, nbs=[3], which="k"),
                lambda: emit_qk_proj(1, nbs=[0], which="q"),
            ]))
            blocks.append((1, 0, 512, [
                lambda: emit_qk_proj(1, nbs=[1], which="q"),
            ]))
            blocks.append((1, 512, 512, [
                lambda: emit_qk_proj(1, nbs=[2], which="q"),
                lambda: emit_outproj([0, 1]),
                lambda: emit_outproj([2, 3]),
            ]))
            blocks.append((1, 1024, 512, [
                lambda: emit_qk_proj(1, nbs=[3], which="q"),
                lambda: emit_outproj([4, 5]),
                lambda: emit_outproj([6, 7]),
            ]))
            blocks.append((1, 1536, 512, [
                lambda: emit_outproj([8, 9]),
                lambda: emit_outproj([10, 11]),
            ]))

            for bi, (hb, q0, qw, dr) in enumerate(blocks):
                drip.extend(dr)
                last = bi == len(blocks) - 1
                fl, fin = emit_attn_pair(hb, q0, qw, hook,
                                         split_last_mult=last)
                if last:
                    fl()
                    fin()
                else:
                    flush_prev = fl
                    finish_prev = fin
            emit_outproj([12], copy_on_act=True)
            emit_outproj([13], copy_on_act=True)
            emit_outproj([14], copy_on_act=True)
            emit_outproj([15], copy_on_act=True)

    nc.compile()
    return nc


def _get_nc():
    global _NC_CACHE
    if _NC_CACHE is None:
        _NC_CACHE = build_nc()
    return _NC_CACHE


def kernel(Q, attn_mask, Wq, bq, Wk, bk, Wv, bv, Wo, bo):
    global LAST_RESULTS
    bf16 = ml_dtypes.bfloat16
    Q = np.asarray(Q, np.float32)
    attn_mask = np.asarray(attn_mask)
    Wq, Wk, Wv, Wo = (np.asarray(w, np.float32) for w in (Wq, Wk, Wv, Wo))
    bq, bk, bv, bo = (np.asarray(b, np.float32) for b in (bq, bk, bv, bo))
    B = Q.shape[0]

    nc = _get_nc()
    in_maps = []
    for c in range(8):
        b, g = c // 4, c % 4
        hs = slice(OC * g, OC * (g + 1))
        in_maps.append(
            {
                "qt": np.ascontiguousarray(Q[b].T).astype(bf16),
                "maskt": np.ascontiguousarray(attn_mask[b, 0].T).astype(bf16),
                "wq": np.ascontiguousarray(Wq[hs].T).astype(bf16),
                "wk": np.ascontiguousarray(Wk[hs].T).astype(bf16),
                "wv": np.ascontiguousarray(Wv[hs].T).astype(bf16),
                "wo": np.ascontiguousarray(Wo[:, hs].T).astype(bf16),
                "bq8": (bq[hs] * 0.125).reshape(OC, 1).astype(np.float32),
                "bk1": bk[hs].reshape(OC, 1).astype(np.float32),
                "bv1": bv[hs].reshape(1, OC).astype(bf16),
            }
        )

    res = run_bass_kernel_spmd(
        nc, in_maps, core_ids=list(range(8)),
        trace=bool(int(os.environ.get("KERNEL_TRACE", "0"))),
    )
    LAST_RESULTS = res
    out = np.zeros((B, S, D), np.float32)
    for c in range(8):
        out[c // 4] += np.asarray(res.results[c]["out"], np.float32)
    out += bo
    return out
